# revision 1
# baseline (speedup 1.0000x reference)
"""Causal self-attention with RoPE on 8 Trainium2 NeuronCores.

Problem: B=4, T=2048, C=1024, 16 heads x 64 dim, fp32 reference.

Sharding: 8 cores = (batch b in 0..3) x (head-group g in 0..1, 8 heads each).
Each core computes qkv for its batch/head-slice (column-parallel qkv),
full attention for its 8 heads, and a row-parallel partial projection.
Host sums the two partial projections per batch (the "all-reduce").

Per-core kernel layout strategy:
  - Host pre-transposes x and weights so every matmul contraction dim is
    on SBUF partitions (fp32 DMA transpose is unsupported on-device).
  - Matmuls run in bf16 (4x faster than fp32 on the PE), fp32 PSUM accum.
  - qkv produced in [t, f] layout; RoPE applied along the free axis
    (fused with PSUM evacuation); q/k then PE-transposed to [d, t] in
    head-pair stacks (2 heads x 64 = 128 partitions).
  - Scores are computed TRANSPOSED: ST[tk, tq] = kT.T @ qT per head
    (two heads run concurrently in the PE array via row tiling).
  - exp on ScalarE straight out of PSUM (scale=1/8 folded in). No max
    subtraction: |scores|/8 < ~40 << 88, safe in fp32/bf16 range.
  - Causal masking: gpsimd affine_select zeroes the upper-triangular
    part of diagonal-straddling exp tiles.
  - attn@v: outT[d, tq] = v.T @ PT with a ones-column appended to v, so
    row 64 of the output accumulates the softmax denominator l for free.
  - Normalization: l broadcast across partitions with a K=1 matmul,
    fast reciprocal on DVE, multiply fused into the PSUM evacuation.
  - proj: row-parallel y_partial = outT.T @ wprojT, fp32 output.
"""

import sys
import threading

sys.path.insert(0, "/opt/trn_rl_repo")

import ml_dtypes
import numpy as np

import concourse.bass as bass
import concourse.mybir as mybir
from concourse import bacc
from concourse.bass_utils import run_bass_kernel_spmd
from concourse.masks import make_identity
from concourse.tile import TileContext

BF16 = ml_dtypes.bfloat16
F32 = mybir.dt.float32
BF = mybir.dt.bfloat16

B, T, C = 4, 2048, 1024
NH, D = 16, 64          # global heads
HL = 8                  # local heads per core
G = 2                   # head groups (cores per batch)
FL = 3 * HL * D         # 1536 local qkv rows
CL = HL * D             # 512 local out channels
P = 128
TQ = 512                # query-block width
NTT = T // P            # 16 t-tiles
NPAIR = HL // 2         # 4 head pairs


def build_nc():
    nc = bacc.Bacc("TRN2", target_bir_lowering=False, debug=False, num_devices=8)

    xT = nc.declare_dram_parameter("xT", [C, T], BF, isOutput=False)
    wqkvT = nc.declare_dram_parameter("wqkvT", [C, FL], BF, isOutput=False)
    wprojT = nc.declare_dram_parameter("wprojT", [CL, C], BF, isOutput=False)
    cos_t = nc.declare_dram_parameter("cos_t", [T, D // 2], F32, isOutput=False)
    msin_t = nc.declare_dram_parameter("msin_t", [T, D // 2], F32, isOutput=False)
    psin_t = nc.declare_dram_parameter("psin_t", [T, D // 2], F32, isOutput=False)
    y = nc.declare_dram_parameter("y", [T, C], F32, isOutput=True)

    Exp = mybir.ActivationFunctionType.Exp

    with TileContext(nc) as tc:
        with (
            tc.tile_pool(name="const", bufs=1) as const,
            tc.tile_pool(name="work", bufs=4) as work,
            tc.tile_pool(name="pt", bufs=6) as ptp,
            tc.tile_pool(name="small", bufs=6) as small,
            tc.tile_pool(name="psmm", bufs=2, space="PSUM") as psmm,
            tc.tile_pool(name="psst", bufs=4, space="PSUM") as psst,
            tc.tile_pool(name="psout", bufs=2, space="PSUM") as psout,
        ):
            # ---- persistent SBUF tensors ----
            XT = const.tile([P, C // P, T], BF, tag="XT")
            WQKV = const.tile([P, C // P, FL], BF, tag="WQKV")
            WPROJ = const.tile([P, CL // P, C], BF, tag="WPROJ")
            COS = const.tile([P, NTT, D // 2], F32, tag="COS")
            MSIN = const.tile([P, NTT, D // 2], F32, tag="MSIN")
            PSIN = const.tile([P, NTT, D // 2], F32, tag="PSIN")
            IDT = const.tile([P, P], BF, tag="IDT")
            ONES1 = const.tile([1, D], BF, tag="ONES1")
            V1 = const.tile([P, NTT, HL, D + 1], BF, tag="V1")
            QT = const.tile([P, NPAIR, T], BF, tag="QT")
            KT = const.tile([P, NPAIR, T], BF, tag="KT")
            ONORM = const.tile([P, NPAIR, T], BF, tag="ONORM")

            xTr = xT.rearrange("(ko p) t -> p ko t", p=P)
            wqr = wqkvT.rearrange("(ko p) f -> p ko f", p=P)
            for ko in range(C // P):  # per-slab loads so compute starts early
                nc.sync.dma_start(XT[:, ko, :], xTr[:, ko, :])
                nc.sync.dma_start(WQKV[:, ko, :], wqr[:, ko, :])
            nc.sync.dma_start(WPROJ[:], wprojT.rearrange("(ko p) o -> p ko o", p=P))
            nc.sync.dma_start(COS[:], cos_t.rearrange("(n p) d -> p n d", p=P))
            nc.sync.dma_start(MSIN[:], msin_t.rearrange("(n p) d -> p n d", p=P))
            nc.sync.dma_start(PSIN[:], psin_t.rearrange("(n p) d -> p n d", p=P))
            make_identity(nc, IDT[:])
            nc.gpsimd.memset(ONES1[:], 1.0)
            nc.gpsimd.memset(V1[:], 1.0)
            MASKS = const.tile([P, TQ // P, TQ], BF, tag="MASKS")
            nc.gpsimd.memset(MASKS[:], 1.0)
            for ml in range(TQ // P):  # keep where q - p - 128*ml >= 0
                nc.gpsimd.affine_select(
                    out=MASKS[:, ml, :], in_=MASKS[:, ml, :],
                    compare_op=mybir.AluOpType.is_ge, fill=0.0,
                    base=-P * ml, pattern=[[1, TQ]], channel_multiplier=-1)

            # ---- fused: qkv/RoPE/transpose interleaved with attention ----
            yr = y.rearrange("(n p) o -> p n o", p=P)
            for i in range(NTT):
                for j3 in range(3):  # 0:q 1:k 2:v
                    ps = psmm.tile([P, 512], F32, tag="mm")
                    for ko in range(C // P):
                        nc.tensor.matmul(
                            ps[:],
                            lhsT=XT[:, ko, i * P:(i + 1) * P],
                            rhs=WQKV[:, ko, j3 * 512:(j3 + 1) * 512],
                            start=(ko == 0),
                            stop=(ko == C // P - 1),
                        )
                    if j3 < 2:
                        ps4 = ps.rearrange("p (h e d) -> p h e d", h=HL, e=2)
                        cosb = COS[:, i, :].unsqueeze(1).unsqueeze(1).to_broadcast(
                            [P, HL, 2, D // 2])
                        msb = MSIN[:, i, :].unsqueeze(1).to_broadcast(
                            [P, HL, D // 2])
                        psb = PSIN[:, i, :].unsqueeze(1).to_broadcast(
                            [P, HL, D // 2])
                        a = work.tile([P, 512], BF, tag="ropeA")
                        a4 = a.rearrange("p (h e d) -> p h e d", h=HL, e=2)
                        b = work.tile([P, 512], BF, tag="ropeB")
                        b4 = b.rearrange("p (h e d) -> p h e d", h=HL, e=2)
                        nc.vector.tensor_mul(a4[:], ps4[:], cosb)
                        nc.vector.tensor_mul(b4[:, :, 0, :], ps4[:, :, 1, :], msb)
                        nc.vector.tensor_mul(b4[:, :, 1, :], ps4[:, :, 0, :], psb)
                        r = work.tile([P, 512], BF, tag="ropeR")
                        nc.gpsimd.tensor_add(r[:], a[:], b[:])
                        dst = QT if j3 == 0 else KT
                        for p4 in range(NPAIR):
                            tp = psst.tile([P, P], BF, tag="st")
                            nc.tensor.transpose(
                                tp[:], r[:, p4 * P:(p4 + 1) * P], IDT[:])
                            nc.vector.tensor_copy(
                                dst[:, p4, i * P:(i + 1) * P], tp[:])
                    else:
                        ps3 = ps.rearrange("p (h d) -> p h d", h=HL)
                        nc.vector.tensor_copy(V1[:, i, :, 0:D], ps3[:])


                if i % (TQ // P) == (TQ // P) - 1:
                    j = i // (TQ // P)
                    ntk = (TQ // P) * (j + 1)
                    qsl = slice(j * TQ, (j + 1) * TQ)
                    for p4 in range(NPAIR):
                        outA = psout.tile([D + 1, TQ], F32, tag="out")
                        outB = psout.tile([D + 1, TQ], F32, tag="out")
                        for m in range(ntk):
                            ksl = slice(m * P, (m + 1) * P)
                            stA = psst.tile([P, TQ], F32, tag="st")
                            stB = psst.tile([P, TQ], F32, tag="st")
                            nc.tensor.matmul(
                                stA[:], lhsT=KT[0:D, p4, ksl], rhs=QT[0:D, p4, qsl],
                                start=True, stop=True)
                            nc.tensor.matmul(
                                stB[:], lhsT=KT[D:P, p4, ksl], rhs=QT[D:P, p4, qsl],
                                start=True, stop=True, tile_position=(D, 0))
                            pA = ptp.tile([P, TQ], BF, tag="ptA")
                            pB = ptp.tile([P, TQ], BF, tag="ptB")
                            nc.scalar.activation(pA[:], stA[:], Exp, scale=0.125)
                            nc.scalar.activation(pB[:], stB[:], Exp, scale=0.125)
                            ml = m - (TQ // P) * j
                            if ml >= 0:  # diagonal-straddling tile: zero tk > tq
                                nc.gpsimd.affine_select(
                                    out=pA[:], in_=pA[:],
                                    compare_op=mybir.AluOpType.is_ge,
                                    fill=0.0, base=-P * ml,
                                    pattern=[[1, TQ]], channel_multiplier=-1)
                                nc.vector.tensor_mul(pB[:], pB[:], MASKS[:, ml, :])
                            nc.tensor.matmul(
                                outA[:], lhsT=V1[:, m, 2 * p4, :], rhs=pA[:],
                                start=(m == 0), stop=(m == ntk - 1))
                            nc.tensor.matmul(
                                outB[:], lhsT=V1[:, m, 2 * p4 + 1, :], rhs=pB[:],
                                start=(m == 0), stop=(m == ntk - 1))
                        for w, outp in ((0, outA), (1, outB)):
                            lrow = small.tile([1, TQ], F32, tag="lrow")
                            nc.vector.tensor_copy(lrow[:], outp[D:D + 1, :])
                            r_row = small.tile([1, TQ], F32, tag="rrow")
                            nc.vector.reciprocal_approx_fast(
                                out=r_row[:], in_=lrow[:])
                            r64 = small.tile([D, TQ], F32, tag="rsb")
                            nc.gpsimd.partition_broadcast(r64[:], r_row[:])
                            nc.vector.tensor_mul(
                                ONORM[w * D:(w + 1) * D, p4, qsl],
                                outp[0:D, :], r64[:])
                    # projection for the query block just finished
                    for i in range((TQ // P) * j, (TQ // P) * (j + 1)):
                        for n2 in range(C // 512):
                            ps = psmm.tile([P, 512], F32, tag="mm")
                            for kc in range(NPAIR):
                                nc.tensor.matmul(
                                    ps[:],
                                    lhsT=ONORM[:, kc, i * P:(i + 1) * P],
                                    rhs=WPROJ[:, kc, n2 * 512:(n2 + 1) * 512],
                                    start=(kc == 0),
                                    stop=(kc == NPAIR - 1),
                                )
                            ysb = work.tile([P, 512], F32, tag="ysb")
                            nc.scalar.copy(out=ysb[:], in_=ps[:])
                            nc.sync.dma_start(yr[:, i, n2 * 512:(n2 + 1) * 512], ysb[:])

    nc.compile()
    return nc


def prep_inputs(x, w_qkv, w_proj):
    """Build the 8 per-core input maps from the full-problem inputs."""
    x = np.asarray(x, dtype=np.float32)
    w_qkv = np.asarray(w_qkv, dtype=np.float32)
    w_proj = np.asarray(w_proj, dtype=np.float32)

    inv_freq = 1.0 / (10000.0 ** (np.arange(0, D, 2, dtype=np.float32) / D))
    tt = np.arange(T, dtype=np.float32)
    freqs = np.outer(tt, inv_freq).astype(np.float32)  # [T, 32]
    cos_t = np.cos(freqs).astype(np.float32)
    sin_t = np.sin(freqs).astype(np.float32)
    msin_t = (-sin_t).astype(np.float32)

    in_maps = []
    for core in range(8):
        b, g = divmod(core, G)
        sl = slice(g * CL, (g + 1) * CL)
        w_local = np.concatenate(
            [w_qkv[sl], w_qkv[C:][sl], w_qkv[2 * C:][sl]], axis=0)  # [1536, C]
        in_maps.append({
            "xT": np.ascontiguousarray(x[b].T).astype(BF16),
            "wqkvT": np.ascontiguousarray(w_local.T).astype(BF16),
            "wprojT": np.ascontiguousarray(w_proj[:, sl].T).astype(BF16),
            "cos_t": cos_t,
            "msin_t": msin_t,
            "psin_t": sin_t,
        })
    return in_maps


_NC_LOCK = threading.Lock()
_NC = None


def get_nc():
    global _NC
    with _NC_LOCK:
        if _NC is None:
            _NC = build_nc()
    return _NC


def run(nc, in_maps, **kw):
    res = run_bass_kernel_spmd(nc, in_maps, list(range(8)), **kw)
    parts = [res.results[c]["y"] for c in range(8)]
    out = np.stack([parts[2 * b] + parts[2 * b + 1] for b in range(B)])
    return out.astype(np.float32), res


def kernel(x, w_qkv, w_proj):
    out, _ = run(get_nc(), prep_inputs(x, w_qkv, w_proj))
    return out



# revision 5
# speedup vs baseline: 1.1295x; 1.1295x over previous
"""Causal self-attention with RoPE on 8 Trainium2 NeuronCores.

Problem: B=4, T=2048, C=1024, 16 heads x 64 dim, fp32 reference.

Sharding: 8 cores = (batch b in 0..3) x (head-group g in 0..1, 8 heads each).
Each core computes qkv for its batch/head-slice (column-parallel qkv),
full attention for its 8 heads, and a row-parallel partial projection.
Host sums the two partial projections per batch (the "all-reduce").

Per-core kernel layout strategy (v2 — no PE transposes):
  - q/k are produced DIRECTLY in [d, t] layout: ps[d_pair(128), t(512)] =
    Wqkv_slice.T @ X, with W as the stationary operand. No transposes.
  - RoPE in [d, t] layout: host permutes W rows so each rotation partner
    (d, d+32) sits +-16 partitions apart inside a 32-partition quadrant;
    DVE stream_shuffle swaps partners, one mul by a sign-folded sin table,
    one mul by cos, gpsimd add -> QT/KT bf16. Scores are invariant to the
    row permutation (same perm on q and k).
  - v stays in [t, f] layout (needed as attn@v stationary operand).
  - Scores TRANSPOSED: ST[tk, tq] = kT.T @ qT; both heads of a pair run
    concurrently via PE row tiling into one 2-bank PSUM group; ONE wide
    exp (ACT) evacuates both banks -> bf16 pAB.
  - Causal trimming: diagonal-straddling key tiles only compute/exp/
    accumulate the valid tq range (saves ~15% PE + ACT).
  - attn@v: outT[d+1, tq] = [V | ones].T @ P; row 64 accumulates the
    softmax denominator l for free.
  - Normalization: fast reciprocal (DVE) of l straight from PSUM,
    gpsimd partition-broadcast, one DVE mul into ONORM bf16.
  - proj: row-parallel y_partial = ONORM.T @ wprojT, fp32 out, ACT evac.
"""

import sys
import threading

sys.path.insert(0, "/opt/trn_rl_repo")

import ml_dtypes
import numpy as np

import concourse.bass as bass
import concourse.mybir as mybir
from concourse import bacc
from concourse.bass_utils import run_bass_kernel_spmd
from concourse.tile import TileContext

BF16 = ml_dtypes.bfloat16
F32 = mybir.dt.float32
BF = mybir.dt.bfloat16

B, T, C = 4, 2048, 1024
NH, D = 16, 64          # global heads
HL = 8                  # local heads per core
G = 2                   # head groups (cores per batch)
CL = HL * D             # 512 local out channels
P = 128
TQ = 512                # query-block width
NTT = T // P            # 16 t-tiles
NPAIR = HL // 2         # 4 head pairs
NKO = C // P            # 8 contraction slabs

# RoPE partner shuffle: within each 32-partition quadrant, swap halves.
SHUF = list(range(16, 32)) + list(range(16))


def build_nc():
    nc = bacc.Bacc("TRN2", target_bir_lowering=False, debug=False, num_devices=8)

    xT = nc.declare_dram_parameter("xT", [C, T], BF, isOutput=False)
    # cols: [pair0 q(128) | pair0 k(128) | pair1 q | ... | v(512)]
    wqkvT = nc.declare_dram_parameter("wqkvT", [C, 1536], BF, isOutput=False)
    wprojT = nc.declare_dram_parameter("wprojT", [CL, C], BF, isOutput=False)
    cos_p = nc.declare_dram_parameter("cos_p", [P, T], F32, isOutput=False)
    sin_s = nc.declare_dram_parameter("sin_s", [P, T], F32, isOutput=False)
    y = nc.declare_dram_parameter("y", [T, C], F32, isOutput=True)

    Exp = mybir.ActivationFunctionType.Exp

    with TileContext(nc) as tc:
        with (
            tc.tile_pool(name="const", bufs=1) as const,
            tc.tile_pool(name="work", bufs=3) as work,
            tc.tile_pool(name="pexp", bufs=3) as pexp,
            tc.tile_pool(name="small", bufs=4) as small,
            tc.tile_pool(name="psmm", bufs=2, space="PSUM") as psmm,
            tc.tile_pool(name="psst", bufs=2, space="PSUM") as psst,
            tc.tile_pool(name="psout", bufs=2, space="PSUM") as psout,
        ):
            # ---- persistent SBUF tensors ----
            XT = const.tile([P, NKO, T], BF, tag="XT")
            WQKV = const.tile([P, NKO, 1536], BF, tag="WQKV")
            WPROJ = const.tile([P, NPAIR, C], BF, tag="WPROJ")
            COS = const.tile([P, T], F32, tag="COS")
            SINS = const.tile([P, T], F32, tag="SINS")
            V1 = const.tile([P, NTT, HL, D + 1], BF, tag="V1")
            QT = const.tile([P, NPAIR, T], BF, tag="QT")
            KT = const.tile([P, NPAIR, T], BF, tag="KT")
            ONORM = const.tile([P, NPAIR, T], BF, tag="ONORM")

            nc.sync.dma_start(COS[:], cos_p[:, :])
            nc.sync.dma_start(SINS[:], sin_s[:, :])
            xTr = xT.rearrange("(ko p) t -> p ko t", p=P)
            wqr = wqkvT.rearrange("(ko p) f -> p ko f", p=P)
            for ko in range(NKO):  # per-slab loads so compute starts early
                nc.sync.dma_start(XT[:, ko, :], xTr[:, ko, :])
                nc.sync.dma_start(WQKV[:, ko, :], wqr[:, ko, :])
            nc.sync.dma_start(WPROJ[:], wprojT.rearrange("(ko p) o -> p ko o", p=P))
            # ones column for the denominator row; v copies fill cols 0:64
            nc.gpsimd.memset(V1[:], 1.0)

            yr = y.rearrange("(n p) o -> p n o", p=P)
            for j in range(T // TQ):
                qsl = slice(j * TQ, (j + 1) * TQ)
                # ---- qkv for block j ----
                for pair in range(NPAIR):
                    for qk in range(2):  # 0:q 1:k
                        ps = psmm.tile([P, TQ], F32, tag="mm")
                        fo = pair * 256 + qk * 128
                        for ko in range(NKO):
                            nc.tensor.matmul(
                                ps[:],
                                lhsT=WQKV[:, ko, fo:fo + 128],
                                rhs=XT[:, ko, qsl],
                                start=(ko == 0),
                                stop=(ko == NKO - 1),
                            )
                        # RoPE: dst = ps*cos + shuffle(ps)*sin_signed
                        sw = work.tile([P, TQ], F32, tag="sw")
                        nc.vector.stream_shuffle(sw[:], ps[:], SHUF)
                        a = work.tile([P, TQ], BF, tag="ra")
                        nc.vector.tensor_mul(a[:], ps[:], COS[:, qsl])
                        b = work.tile([P, TQ], BF, tag="rb")
                        nc.vector.tensor_mul(b[:], sw[:], SINS[:, qsl])
                        dst = QT if qk == 0 else KT
                        nc.gpsimd.tensor_add(dst[:, pair, qsl], a[:], b[:])
                for i in range(4 * j, 4 * j + 4):  # v tiles
                    ps = psmm.tile([P, TQ], F32, tag="mm")
                    for ko in range(NKO):
                        nc.tensor.matmul(
                            ps[:],
                            lhsT=XT[:, ko, i * P:(i + 1) * P],
                            rhs=WQKV[:, ko, 1024:1536],
                            start=(ko == 0),
                            stop=(ko == NKO - 1),
                        )
                        pass
                    nc.vector.tensor_copy(
                        V1[:, i, :, 0:D],
                        ps.rearrange("p (h d) -> p h d", h=HL),
                    )
                # ---- attention for block j ----
                ntk = 4 * (j + 1)
                for pair in range(NPAIR):
                    outA = psout.tile([D + 1, TQ], F32, tag="out")
                    outB = psout.tile([D + 1, TQ], F32, tag="out")
                    for m in range(ntk):
                        ml = m - 4 * j
                        off = P * max(ml, 0)
                        w = TQ - off
                        ksl = slice(m * P, (m + 1) * P)
                        tsl = slice(j * TQ + off, (j + 1) * TQ)
                        st = psst.tile([P, 2, TQ], F32, tag="st")
                        nc.tensor.matmul(
                            st[:, 0, off:], lhsT=KT[0:D, pair, ksl],
                            rhs=QT[0:D, pair, tsl], start=True, stop=True)
                        nc.tensor.matmul(
                            st[:, 1, off:], lhsT=KT[D:P, pair, ksl],
                            rhs=QT[D:P, pair, tsl],
                            start=True, stop=True, tile_position=(D, 0))
                        pAB = pexp.tile([P, 2, TQ], BF, tag="p")
                        nc.scalar.activation(
                            pAB[:, :, off:], st[:, :, off:], Exp, scale=0.125)
                        if ml >= 0:  # diagonal tile: zero tk > tq
                            for g in range(2):
                                nc.gpsimd.affine_select(
                                    out=pAB[:, g, off:], in_=pAB[:, g, off:],
                                    compare_op=mybir.AluOpType.is_ge,
                                    fill=0.0, base=0,
                                    pattern=[[1, w]], channel_multiplier=-1)
                        nc.tensor.matmul(
                            outA[:, off:], lhsT=V1[:, m, 2 * pair, :],
                            rhs=pAB[:, 0, off:],
                            start=(m == 0), stop=(m == ntk - 1))
                        nc.tensor.matmul(
                            outB[:, off:], lhsT=V1[:, m, 2 * pair + 1, :],
                            rhs=pAB[:, 1, off:],
                            start=(m == 0), stop=(m == ntk - 1))
                    for hw, outp in ((0, outA), (1, outB)):
                        lrow = small.tile([1, TQ], F32, tag="lr")
                        nc.vector.tensor_copy(lrow[:], outp[D:D + 1, :])
                        rrow = small.tile([1, TQ], F32, tag="rr")
                        nc.vector.reciprocal_approx_fast(
                            out=rrow[:], in_=lrow[:])
                        r64 = small.tile([D, TQ], F32, tag="r64")
                        nc.gpsimd.partition_broadcast(r64[:], rrow[:])
                        nc.vector.tensor_mul(
                            ONORM[hw * D:(hw + 1) * D, pair, qsl],
                            outp[0:D, :], r64[:])
                # ---- proj for block j ----
                for i in range(4 * j, 4 * j + 4):
                    for n2 in range(2):
                        ps = psmm.tile([P, TQ], F32, tag="mm")
                        for kc in range(NPAIR):
                            nc.tensor.matmul(
                                ps[:],
                                lhsT=ONORM[:, kc, i * P:(i + 1) * P],
                                rhs=WPROJ[:, kc, n2 * 512:(n2 + 1) * 512],
                                start=(kc == 0),
                                stop=(kc == NPAIR - 1),
                            )
                        ysb = work.tile([P, TQ], F32, tag="ysb")
                        nc.scalar.copy(out=ysb[:], in_=ps[:])
                        nc.sync.dma_start(yr[:, i, n2 * 512:(n2 + 1) * 512], ysb[:])

    nc.compile()
    return nc


def _rope_perm():
    """perm[d] = partition row for head-local dim d (0..63)."""
    perm = np.zeros(D, dtype=np.int64)
    for d in range(32):          # q1 half
        perm[d] = 32 * (d // 16) + (d % 16)
    for d in range(32, 64):      # q2 half
        dd = d - 32
        perm[d] = 32 * (dd // 16) + 16 + (dd % 16)
    return perm


def prep_inputs(x, w_qkv, w_proj):
    """Build the 8 per-core input maps from the full-problem inputs."""
    x = np.asarray(x, dtype=np.float32)
    w_qkv = np.asarray(w_qkv, dtype=np.float32)
    w_proj = np.asarray(w_proj, dtype=np.float32)

    perm = _rope_perm()
    inv = np.argsort(perm)  # inv[p] = original d

    # RoPE tables in [partition(128), t] layout, matching the row perm.
    inv_freq = 1.0 / (10000.0 ** (np.arange(0, D, 2, dtype=np.float32) / D))
    tt = np.arange(T, dtype=np.float32)
    p_idx = np.arange(P)
    fi = 16 * ((p_idx // 32) % 2) + (p_idx % 16)      # freq index per row
    sign = np.where(p_idx % 32 < 16, -1.0, 1.0).astype(np.float32)
    ang = np.outer(inv_freq[fi], tt)                   # [128, T]
    cos_p = np.cos(ang).astype(np.float32)
    sin_s = (sign[:, None] * np.sin(ang)).astype(np.float32)

    in_maps = []
    for core in range(8):
        b, g = divmod(core, G)
        sl = slice(g * CL, (g + 1) * CL)
        wq = w_qkv[sl]              # [512, C] local q rows
        wk = w_qkv[C:][sl]
        wv = w_qkv[2 * C:][sl]
        blocks = []
        for pair in range(NPAIR):
            for wmat in (wq, wk):
                blk = wmat[2 * pair * D:(2 * pair + 2) * D]  # [128, C]
                blk = blk.reshape(2, D, C)[:, inv, :].reshape(128, C)
                blocks.append(blk)
        blocks.append(wv)
        w_local = np.concatenate(blocks, axis=0)       # [1536, C]
        in_maps.append({
            "xT": np.ascontiguousarray(x[b].T).astype(BF16),
            "wqkvT": np.ascontiguousarray(w_local.T).astype(BF16),
            "wprojT": np.ascontiguousarray(w_proj[:, sl].T).astype(BF16),
            "cos_p": cos_p,
            "sin_s": sin_s,
        })
    return in_maps


_NC_LOCK = threading.Lock()
_NC = None


def get_nc():
    global _NC
    with _NC_LOCK:
        if _NC is None:
            _NC = build_nc()
    return _NC


def run(nc, in_maps, **kw):
    res = run_bass_kernel_spmd(nc, in_maps, list(range(8)), **kw)
    parts = [res.results[c]["y"] for c in range(8)]
    out = np.stack([parts[2 * b] + parts[2 * b + 1] for b in range(B)])
    return out.astype(np.float32), res


def kernel(x, w_qkv, w_proj):
    out, _ = run(get_nc(), prep_inputs(x, w_qkv, w_proj))
    return out


# revision 8
# speedup vs baseline: 1.4051x; 1.2440x over previous
"""Causal self-attention with RoPE on 8 Trainium2 NeuronCores.

Problem: B=4, T=2048, C=1024, 16 heads x 64 dim, fp32 reference.

Sharding: 8 cores = (batch b in 0..3) x (head-group g in 0..1, 8 heads each).
Each core computes qkv for its batch/head-slice (column-parallel qkv),
full attention for its 8 heads, and a row-parallel partial projection.
Host sums the two partial projections per batch (the "all-reduce").

Per-core kernel layout strategy (v2 — no PE transposes):
  - q/k are produced DIRECTLY in [d, t] layout: ps[d_pair(128), t(512)] =
    Wqkv_slice.T @ X, with W as the stationary operand. No transposes.
  - RoPE in [d, t] layout: host permutes W rows so each rotation partner
    (d, d+32) sits +-16 partitions apart inside a 32-partition quadrant;
    DVE stream_shuffle swaps partners, one mul by a sign-folded sin table,
    one mul by cos, gpsimd add -> QT/KT bf16. Scores are invariant to the
    row permutation (same perm on q and k).
  - v stays in [t, f] layout (needed as attn@v stationary operand).
  - Scores TRANSPOSED: ST[tk, tq] = kT.T @ qT; both heads of a pair run
    concurrently via PE row tiling into one 2-bank PSUM group; ONE wide
    exp (ACT) evacuates both banks -> bf16 pAB.
  - Causal trimming: diagonal-straddling key tiles only compute/exp/
    accumulate the valid tq range (saves ~15% PE + ACT).
  - attn@v: outT[d+1, tq] = [V | ones].T @ P; row 64 accumulates the
    softmax denominator l for free.
  - Normalization: fast reciprocal (DVE) of l straight from PSUM,
    gpsimd partition-broadcast, one DVE mul into ONORM bf16.
  - proj: row-parallel y_partial = ONORM.T @ wprojT, fp32 out, ACT evac.
"""

import sys
import threading

sys.path.insert(0, "/opt/trn_rl_repo")

import ml_dtypes
import numpy as np

import concourse.bass as bass
import concourse.mybir as mybir
from concourse import bacc
from concourse.bass_utils import run_bass_kernel_spmd
from concourse.tile import TileContext

BF16 = ml_dtypes.bfloat16
F32 = mybir.dt.float32
BF = mybir.dt.bfloat16

B, T, C = 4, 2048, 1024
NH, D = 16, 64          # global heads
HL = 8                  # local heads per core
G = 2                   # head groups (cores per batch)
CL = HL * D             # 512 local out channels
P = 128
TQ = 512                # query-block width
NTT = T // P            # 16 t-tiles
NPAIR = HL // 2         # 4 head pairs
NKO = C // P            # 8 contraction slabs

# RoPE partner shuffle: within each 32-partition quadrant, swap halves.
SHUF = list(range(16, 32)) + list(range(16))


def build_nc():
    nc = bacc.Bacc("TRN2", target_bir_lowering=False, debug=False, num_devices=8)

    xT = nc.declare_dram_parameter("xT", [C, T], BF, isOutput=False)
    # cols: [pair0 q(128) | pair0 k(128) | pair1 q | ... | v(512)]
    wqkvT = nc.declare_dram_parameter("wqkvT", [C, 1536], BF, isOutput=False)
    wprojT = nc.declare_dram_parameter("wprojT", [CL, C], BF, isOutput=False)
    cos_p = nc.declare_dram_parameter("cos_p", [P, T], F32, isOutput=False)
    sin_s = nc.declare_dram_parameter("sin_s", [P, T], F32, isOutput=False)
    y = nc.declare_dram_parameter("y", [T, C], F32, isOutput=True)

    Exp = mybir.ActivationFunctionType.Exp

    with TileContext(nc) as tc:
        with (
            tc.tile_pool(name="const", bufs=1) as const,
            tc.tile_pool(name="work", bufs=3) as work,
            tc.tile_pool(name="pexp", bufs=4) as pexp,
            tc.tile_pool(name="small", bufs=4) as small,
            tc.tile_pool(name="psmm", bufs=2, space="PSUM") as psmm,
            tc.tile_pool(name="psst", bufs=2, space="PSUM") as psst,
            tc.tile_pool(name="psout", bufs=2, space="PSUM") as psout,
        ):
            # ---- persistent SBUF tensors ----
            XT = const.tile([P, NKO, T], BF, tag="XT")
            WQKV = const.tile([P, NKO, 1536], BF, tag="WQKV")
            WPROJ = const.tile([P, NPAIR, C], BF, tag="WPROJ")
            COS = const.tile([P, T], F32, tag="COS")
            SINS = const.tile([P, T], F32, tag="SINS")
            V1 = const.tile([P, NTT, HL, D + 1], BF, tag="V1")
            QT = const.tile([P, NPAIR, T], BF, tag="QT")
            KT = const.tile([P, NPAIR, T], BF, tag="KT")
            ONORM = const.tile([P, NPAIR, T], BF, tag="ONORM")

            nc.sync.dma_start(COS[:], cos_p[:, :])
            nc.sync.dma_start(SINS[:], sin_s[:, :])
            xTr = xT.rearrange("(ko p) t -> p ko t", p=P)
            wqr = wqkvT.rearrange("(ko p) f -> p ko f", p=P)
            for ko in range(NKO):  # per-slab loads so compute starts early
                nc.sync.dma_start(XT[:, ko, :], xTr[:, ko, :])
                nc.sync.dma_start(WQKV[:, ko, :], wqr[:, ko, :])
            nc.sync.dma_start(WPROJ[:], wprojT.rearrange("(ko p) o -> p ko o", p=P))
            # ones column for the denominator row; v copies fill cols 0:64
            nc.gpsimd.memset(V1[:], 1.0)

            yr = y.rearrange("(n p) o -> p n o", p=P)
            def emit_proj(j):
                for i in range(4 * j, 4 * j + 4):
                    for n2 in range(2):
                        ps = psmm.tile([P, TQ], F32, tag="mm")
                        for kc in range(NPAIR):
                            nc.tensor.matmul(
                                ps[:],
                                lhsT=ONORM[:, kc, i * P:(i + 1) * P],
                                rhs=WPROJ[:, kc, n2 * 512:(n2 + 1) * 512],
                                start=(kc == 0),
                                stop=(kc == NPAIR - 1),
                            )
                        ysb = work.tile([P, TQ], F32, tag="ysb")
                        nc.scalar.copy(out=ysb[:], in_=ps[:])
                        nc.sync.dma_start(yr[:, i, n2 * 512:(n2 + 1) * 512], ysb[:])

            for j in range(T // TQ):
                qsl = slice(j * TQ, (j + 1) * TQ)
                # ---- qkv for block j: v first (attn@v waits on V copies) ----
                for i in range(4 * j, 4 * j + 4):  # v tiles
                    ps = psmm.tile([P, TQ], F32, tag="mm")
                    for ko in range(NKO):
                        nc.tensor.matmul(
                            ps[:],
                            lhsT=XT[:, ko, i * P:(i + 1) * P],
                            rhs=WQKV[:, ko, 1024:1536],
                            start=(ko == 0),
                            stop=(ko == NKO - 1),
                        )
                    nc.vector.tensor_copy(
                        V1[:, i, :, 0:D],
                        ps.rearrange("p (h d) -> p h d", h=HL),
                    )
                for pair in range(NPAIR):
                    for qk in range(2):  # 0:q 1:k
                        ps = psmm.tile([P, TQ], F32, tag="mm")
                        fo = pair * 256 + qk * 128
                        for ko in range(NKO):
                            nc.tensor.matmul(
                                ps[:],
                                lhsT=WQKV[:, ko, fo:fo + 128],
                                rhs=XT[:, ko, qsl],
                                start=(ko == 0),
                                stop=(ko == NKO - 1),
                            )
                        # RoPE: dst = ps*cos + shuffle(ps)*sin_signed
                        sw = work.tile([P, TQ], F32, tag="sw")
                        nc.vector.stream_shuffle(sw[:], ps[:], SHUF)
                        a = work.tile([P, TQ], BF, tag="ra")
                        nc.vector.tensor_mul(a[:], ps[:], COS[:, qsl])
                        b = work.tile([P, TQ], BF, tag="rb")
                        nc.vector.tensor_mul(b[:], sw[:], SINS[:, qsl])
                        dst = QT if qk == 0 else KT
                        nc.vector.tensor_add(dst[:, pair, qsl], a[:], b[:])
                if j > 0:  # software-pipelined: proj of the previous block
                    emit_proj(j - 1)
                # ---- attention for block j ----
                ntk = 4 * (j + 1)
                for pair in range(NPAIR):
                    outA = psout.tile([D + 1, TQ], F32, tag="out")
                    outB = psout.tile([D + 1, TQ], F32, tag="out")
                    for m in range(ntk):
                        ml = m - 4 * j
                        off = P * max(ml, 0)
                        w = TQ - off
                        ksl = slice(m * P, (m + 1) * P)
                        tsl = slice(j * TQ + off, (j + 1) * TQ)
                        st = psst.tile([P, 2, TQ], F32, tag="st")
                        nc.tensor.matmul(
                            st[:, 0, off:], lhsT=KT[0:D, pair, ksl],
                            rhs=QT[0:D, pair, tsl], start=True, stop=True)
                        nc.tensor.matmul(
                            st[:, 1, off:], lhsT=KT[D:P, pair, ksl],
                            rhs=QT[D:P, pair, tsl],
                            start=True, stop=True, tile_position=(D, 0))
                        pAB = pexp.tile([P, 2, TQ], BF, tag="p")
                        nc.scalar.activation(
                            pAB[:, :, off:], st[:, :, off:], Exp, scale=0.125)
                        if ml >= 0:  # diagonal tile: zero tk > tq
                            for g in range(2):
                                nc.gpsimd.affine_select(
                                    out=pAB[:, g, off:], in_=pAB[:, g, off:],
                                    compare_op=mybir.AluOpType.is_ge,
                                    fill=0.0, base=0,
                                    pattern=[[1, w]], channel_multiplier=-1)
                        nc.tensor.matmul(
                            outA[:, off:], lhsT=V1[:, m, 2 * pair, :],
                            rhs=pAB[:, 0, off:],
                            start=(m == 0), stop=(m == ntk - 1))
                        nc.tensor.matmul(
                            outB[:, off:], lhsT=V1[:, m, 2 * pair + 1, :],
                            rhs=pAB[:, 1, off:],
                            start=(m == 0), stop=(m == ntk - 1))
                    for hw, outp in ((0, outA), (1, outB)):
                        lrow = small.tile([1, TQ], F32, tag="lr")
                        nc.vector.tensor_copy(lrow[:], outp[D:D + 1, :])
                        rrow = small.tile([1, TQ], F32, tag="rr")
                        nc.vector.reciprocal_approx_fast(
                            out=rrow[:], in_=lrow[:])
                        r64 = small.tile([D, TQ], F32, tag="r64")
                        nc.gpsimd.partition_broadcast(r64[:], rrow[:])
                        nc.vector.tensor_mul(
                            ONORM[hw * D:(hw + 1) * D, pair, qsl],
                            outp[0:D, :], r64[:])
            emit_proj(T // TQ - 1)  # drain the software pipeline

    nc.compile()
    return nc


def _rope_perm():
    """perm[d] = partition row for head-local dim d (0..63)."""
    perm = np.zeros(D, dtype=np.int64)
    for d in range(32):          # q1 half
        perm[d] = 32 * (d // 16) + (d % 16)
    for d in range(32, 64):      # q2 half
        dd = d - 32
        perm[d] = 32 * (dd // 16) + 16 + (dd % 16)
    return perm


def prep_inputs(x, w_qkv, w_proj):
    """Build the 8 per-core input maps from the full-problem inputs."""
    x = np.asarray(x, dtype=np.float32)
    w_qkv = np.asarray(w_qkv, dtype=np.float32)
    w_proj = np.asarray(w_proj, dtype=np.float32)

    perm = _rope_perm()
    inv = np.argsort(perm)  # inv[p] = original d

    # RoPE tables in [partition(128), t] layout, matching the row perm.
    inv_freq = 1.0 / (10000.0 ** (np.arange(0, D, 2, dtype=np.float32) / D))
    tt = np.arange(T, dtype=np.float32)
    p_idx = np.arange(P)
    fi = 16 * ((p_idx // 32) % 2) + (p_idx % 16)      # freq index per row
    sign = np.where(p_idx % 32 < 16, -1.0, 1.0).astype(np.float32)
    ang = np.outer(inv_freq[fi], tt)                   # [128, T]
    cos_p = np.cos(ang).astype(np.float32)
    sin_s = (sign[:, None] * np.sin(ang)).astype(np.float32)

    in_maps = []
    for core in range(8):
        b, g = divmod(core, G)
        sl = slice(g * CL, (g + 1) * CL)
        wq = w_qkv[sl]              # [512, C] local q rows
        wk = w_qkv[C:][sl]
        wv = w_qkv[2 * C:][sl]
        blocks = []
        for pair in range(NPAIR):
            for wmat in (wq, wk):
                blk = wmat[2 * pair * D:(2 * pair + 2) * D]  # [128, C]
                blk = blk.reshape(2, D, C)[:, inv, :].reshape(128, C)
                blocks.append(blk)
        blocks.append(wv)
        w_local = np.concatenate(blocks, axis=0)       # [1536, C]
        in_maps.append({
            "xT": np.ascontiguousarray(x[b].T).astype(BF16),
            "wqkvT": np.ascontiguousarray(w_local.T).astype(BF16),
            "wprojT": np.ascontiguousarray(w_proj[:, sl].T).astype(BF16),
            "cos_p": cos_p,
            "sin_s": sin_s,
        })
    return in_maps


_NC_LOCK = threading.Lock()
_NC = None


def get_nc():
    global _NC
    with _NC_LOCK:
        if _NC is None:
            _NC = build_nc()
    return _NC


def run(nc, in_maps, **kw):
    res = run_bass_kernel_spmd(nc, in_maps, list(range(8)), **kw)
    parts = [res.results[c]["y"] for c in range(8)]
    out = np.stack([parts[2 * b] + parts[2 * b + 1] for b in range(B)])
    return out.astype(np.float32), res


def kernel(x, w_qkv, w_proj):
    out, _ = run(get_nc(), prep_inputs(x, w_qkv, w_proj))
    return out


# revision 10
# speedup vs baseline: 1.4068x; 1.0012x over previous
"""Causal self-attention with RoPE on 8 Trainium2 NeuronCores.

Problem: B=4, T=2048, C=1024, 16 heads x 64 dim, fp32 reference.

Sharding: 8 cores = (batch b in 0..3) x (head-group g in 0..1, 8 heads each).
Each core computes qkv for its batch/head-slice (column-parallel qkv),
full attention for its 8 heads, and a row-parallel partial projection.
Host sums the two partial projections per batch (the "all-reduce").

Per-core kernel layout strategy (v2 — no PE transposes):
  - q/k are produced DIRECTLY in [d, t] layout: ps[d_pair(128), t(512)] =
    Wqkv_slice.T @ X, with W as the stationary operand. No transposes.
  - RoPE in [d, t] layout: host permutes W rows so each rotation partner
    (d, d+32) sits +-16 partitions apart inside a 32-partition quadrant;
    DVE stream_shuffle swaps partners, one mul by a sign-folded sin table,
    one mul by cos, gpsimd add -> QT/KT bf16. Scores are invariant to the
    row permutation (same perm on q and k).
  - v stays in [t, f] layout (needed as attn@v stationary operand).
  - Scores TRANSPOSED: ST[tk, tq] = kT.T @ qT; both heads of a pair run
    concurrently via PE row tiling into one 2-bank PSUM group; ONE wide
    exp (ACT) evacuates both banks -> bf16 pAB.
  - Causal trimming: diagonal-straddling key tiles only compute/exp/
    accumulate the valid tq range (saves ~15% PE + ACT).
  - attn@v: outT[d+1, tq] = [V | ones].T @ P; row 64 accumulates the
    softmax denominator l for free.
  - Normalization: fast reciprocal (DVE) of l straight from PSUM,
    gpsimd partition-broadcast, one DVE mul into ONORM bf16.
  - proj: row-parallel y_partial = ONORM.T @ wprojT, fp32 out, ACT evac.
"""

import sys
import threading

sys.path.insert(0, "/opt/trn_rl_repo")

import ml_dtypes
import numpy as np

import concourse.bass as bass
import concourse.mybir as mybir
from concourse import bacc
from concourse.bass_utils import run_bass_kernel_spmd
from concourse.tile import TileContext

BF16 = ml_dtypes.bfloat16
F32 = mybir.dt.float32
BF = mybir.dt.bfloat16

B, T, C = 4, 2048, 1024
NH, D = 16, 64          # global heads
HL = 8                  # local heads per core
G = 2                   # head groups (cores per batch)
CL = HL * D             # 512 local out channels
P = 128
TQ = 512                # query-block width
NTT = T // P            # 16 t-tiles
NPAIR = HL // 2         # 4 head pairs
NKO = C // P            # 8 contraction slabs

# RoPE partner shuffle: within each 32-partition quadrant, swap halves.
SHUF = list(range(16, 32)) + list(range(16))


def build_nc():
    nc = bacc.Bacc("TRN2", target_bir_lowering=False, debug=False, num_devices=8)

    xT = nc.declare_dram_parameter("xT", [C, T], BF, isOutput=False)
    # cols: [pair0 q(128) | pair0 k(128) | pair1 q | ... | v(512)]
    wqkvT = nc.declare_dram_parameter("wqkvT", [C, 1536], BF, isOutput=False)
    wprojT = nc.declare_dram_parameter("wprojT", [CL, C], BF, isOutput=False)
    cos_p = nc.declare_dram_parameter("cos_p", [P, T], F32, isOutput=False)
    sin_s = nc.declare_dram_parameter("sin_s", [P, T], F32, isOutput=False)
    y = nc.declare_dram_parameter("y", [T, C], F32, isOutput=True)

    Exp = mybir.ActivationFunctionType.Exp

    with TileContext(nc) as tc:
        with (
            tc.tile_pool(name="const", bufs=1) as const,
            tc.tile_pool(name="work", bufs=3) as work,
            tc.tile_pool(name="pexp", bufs=4) as pexp,
            tc.tile_pool(name="small", bufs=4) as small,
            tc.tile_pool(name="psmm", bufs=2, space="PSUM") as psmm,
            tc.tile_pool(name="psst", bufs=2, space="PSUM") as psst,
            tc.tile_pool(name="psout", bufs=2, space="PSUM") as psout,
        ):
            # ---- persistent SBUF tensors ----
            XT = const.tile([P, NKO, T], BF, tag="XT")
            WQKV = const.tile([P, NKO, 1536], BF, tag="WQKV")
            WPROJ = const.tile([P, NPAIR, C], BF, tag="WPROJ")
            COS = const.tile([P, T], F32, tag="COS")
            SINS = const.tile([P, T], F32, tag="SINS")
            V1 = const.tile([P, NTT, HL, D + 1], BF, tag="V1")
            QT = const.tile([P, NPAIR, T], BF, tag="QT")
            KT = const.tile([P, NPAIR, T], BF, tag="KT")
            ONORM = const.tile([P, NPAIR, T], BF, tag="ONORM")

            xTr = xT.rearrange("(ko p) t -> p ko t", p=P)
            wqr = wqkvT.rearrange("(ko p) f -> p ko f", p=P)
            for ko in range(NKO):  # per-slab loads so compute starts early
                nc.sync.dma_start(XT[:, ko, :], xTr[:, ko, :])
                nc.sync.dma_start(WQKV[:, ko, :], wqr[:, ko, :])
                if ko == 0:
                    nc.sync.dma_start(COS[:], cos_p[:, :])
                    nc.sync.dma_start(SINS[:], sin_s[:, :])
            nc.sync.dma_start(WPROJ[:], wprojT.rearrange("(ko p) o -> p ko o", p=P))
            # ones column for the denominator row; v copies fill cols 0:64
            nc.gpsimd.memset(V1[:, :, :, D:D + 1], 1.0)

            yr = y.rearrange("(n p) o -> p n o", p=P)
            def emit_proj(j):
                for i in range(4 * j, 4 * j + 4):
                    for n2 in range(2):
                        ps = psmm.tile([P, TQ], F32, tag="mm")
                        for kc in range(NPAIR):
                            nc.tensor.matmul(
                                ps[:],
                                lhsT=ONORM[:, kc, i * P:(i + 1) * P],
                                rhs=WPROJ[:, kc, n2 * 512:(n2 + 1) * 512],
                                start=(kc == 0),
                                stop=(kc == NPAIR - 1),
                            )
                        ysb = work.tile([P, TQ], F32, tag="ysb")
                        nc.vector.tensor_copy(ysb[:], ps[:])
                        nc.sync.dma_start(yr[:, i, n2 * 512:(n2 + 1) * 512], ysb[:])

            for j in range(T // TQ):
                qsl = slice(j * TQ, (j + 1) * TQ)
                # ---- qkv for block j: v first (attn@v waits on V copies) ----
                for i in range(4 * j, 4 * j + 4):  # v tiles
                    ps = psmm.tile([P, TQ], F32, tag="mm")
                    for ko in range(NKO):
                        nc.tensor.matmul(
                            ps[:],
                            lhsT=XT[:, ko, i * P:(i + 1) * P],
                            rhs=WQKV[:, ko, 1024:1536],
                            start=(ko == 0),
                            stop=(ko == NKO - 1),
                        )
                    nc.vector.tensor_copy(
                        V1[:, i, :, 0:D],
                        ps.rearrange("p (h d) -> p h d", h=HL),
                    )
                for pair in range(NPAIR):
                    for qk in range(2):  # 0:q 1:k
                        ps = psmm.tile([P, TQ], F32, tag="mm")
                        fo = pair * 256 + qk * 128
                        for ko in range(NKO):
                            nc.tensor.matmul(
                                ps[:],
                                lhsT=WQKV[:, ko, fo:fo + 128],
                                rhs=XT[:, ko, qsl],
                                start=(ko == 0),
                                stop=(ko == NKO - 1),
                            )
                        # RoPE: dst = ps*cos + shuffle(ps)*sin_signed
                        sw = work.tile([P, TQ], F32, tag="sw")
                        nc.vector.stream_shuffle(sw[:], ps[:], SHUF)
                        a = work.tile([P, TQ], BF, tag="ra")
                        nc.vector.tensor_mul(a[:], ps[:], COS[:, qsl])
                        b = work.tile([P, TQ], BF, tag="rb")
                        nc.vector.tensor_mul(b[:], sw[:], SINS[:, qsl])
                        dst = QT if qk == 0 else KT
                        nc.vector.tensor_add(dst[:, pair, qsl], a[:], b[:])
                if j > 0:  # software-pipelined: proj of the previous block
                    emit_proj(j - 1)
                # ---- attention for block j ----
                ntk = 4 * (j + 1)
                for pair in range(NPAIR):
                    outA = psout.tile([D + 1, TQ], F32, tag="out")
                    outB = psout.tile([D + 1, TQ], F32, tag="out")
                    for m in range(ntk):
                        ml = m - 4 * j
                        off = P * max(ml, 0)
                        w = TQ - off
                        ksl = slice(m * P, (m + 1) * P)
                        tsl = slice(j * TQ + off, (j + 1) * TQ)
                        st = psst.tile([P, 2, TQ], F32, tag="st")
                        nc.tensor.matmul(
                            st[:, 0, off:], lhsT=KT[0:D, pair, ksl],
                            rhs=QT[0:D, pair, tsl], start=True, stop=True)
                        nc.tensor.matmul(
                            st[:, 1, off:], lhsT=KT[D:P, pair, ksl],
                            rhs=QT[D:P, pair, tsl],
                            start=True, stop=True, tile_position=(D, 0))
                        pAB = pexp.tile([P, 2, TQ], BF, tag="p")
                        nc.scalar.activation(
                            pAB[:, :, off:], st[:, :, off:], Exp, scale=0.125)
                        if ml >= 0:  # diagonal tile: zero tk > tq
                            for g in range(2):
                                nc.gpsimd.affine_select(
                                    out=pAB[:, g, off:], in_=pAB[:, g, off:],
                                    compare_op=mybir.AluOpType.is_ge,
                                    fill=0.0, base=0,
                                    pattern=[[1, w]], channel_multiplier=-1)
                        nc.tensor.matmul(
                            outA[:, off:], lhsT=V1[:, m, 2 * pair, :],
                            rhs=pAB[:, 0, off:],
                            start=(m == 0), stop=(m == ntk - 1))
                        nc.tensor.matmul(
                            outB[:, off:], lhsT=V1[:, m, 2 * pair + 1, :],
                            rhs=pAB[:, 1, off:],
                            start=(m == 0), stop=(m == ntk - 1))
                    for hw, outp in ((0, outA), (1, outB)):
                        lrow = small.tile([1, TQ], F32, tag="lr")
                        nc.vector.tensor_copy(lrow[:], outp[D:D + 1, :])
                        rrow = small.tile([1, TQ], F32, tag="rr")
                        nc.vector.reciprocal_approx_fast(
                            out=rrow[:], in_=lrow[:])
                        r64 = small.tile([D, TQ], F32, tag="r64")
                        nc.gpsimd.partition_broadcast(r64[:], rrow[:])
                        nc.vector.tensor_mul(
                            ONORM[hw * D:(hw + 1) * D, pair, qsl],
                            outp[0:D, :], r64[:])
            emit_proj(T // TQ - 1)  # drain the software pipeline

    nc.compile()
    return nc


def _rope_perm():
    """perm[d] = partition row for head-local dim d (0..63)."""
    perm = np.zeros(D, dtype=np.int64)
    for d in range(32):          # q1 half
        perm[d] = 32 * (d // 16) + (d % 16)
    for d in range(32, 64):      # q2 half
        dd = d - 32
        perm[d] = 32 * (dd // 16) + 16 + (dd % 16)
    return perm


def prep_inputs(x, w_qkv, w_proj):
    """Build the 8 per-core input maps from the full-problem inputs."""
    x = np.asarray(x, dtype=np.float32)
    w_qkv = np.asarray(w_qkv, dtype=np.float32)
    w_proj = np.asarray(w_proj, dtype=np.float32)

    perm = _rope_perm()
    inv = np.argsort(perm)  # inv[p] = original d

    # RoPE tables in [partition(128), t] layout, matching the row perm.
    inv_freq = 1.0 / (10000.0 ** (np.arange(0, D, 2, dtype=np.float32) / D))
    tt = np.arange(T, dtype=np.float32)
    p_idx = np.arange(P)
    fi = 16 * ((p_idx // 32) % 2) + (p_idx % 16)      # freq index per row
    sign = np.where(p_idx % 32 < 16, -1.0, 1.0).astype(np.float32)
    ang = np.outer(inv_freq[fi], tt)                   # [128, T]
    cos_p = np.cos(ang).astype(np.float32)
    sin_s = (sign[:, None] * np.sin(ang)).astype(np.float32)

    in_maps = []
    for core in range(8):
        b, g = divmod(core, G)
        sl = slice(g * CL, (g + 1) * CL)
        wq = w_qkv[sl]              # [512, C] local q rows
        wk = w_qkv[C:][sl]
        wv = w_qkv[2 * C:][sl]
        blocks = []
        for pair in range(NPAIR):
            for wmat in (wq, wk):
                blk = wmat[2 * pair * D:(2 * pair + 2) * D]  # [128, C]
                blk = blk.reshape(2, D, C)[:, inv, :].reshape(128, C)
                blocks.append(blk)
        blocks.append(wv)
        w_local = np.concatenate(blocks, axis=0)       # [1536, C]
        in_maps.append({
            "xT": np.ascontiguousarray(x[b].T).astype(BF16),
            "wqkvT": np.ascontiguousarray(w_local.T).astype(BF16),
            "wprojT": np.ascontiguousarray(w_proj[:, sl].T).astype(BF16),
            "cos_p": cos_p,
            "sin_s": sin_s,
        })
    return in_maps


_NC_LOCK = threading.Lock()
_NC = None


def get_nc():
    global _NC
    with _NC_LOCK:
        if _NC is None:
            _NC = build_nc()
    return _NC


def run(nc, in_maps, **kw):
    res = run_bass_kernel_spmd(nc, in_maps, list(range(8)), **kw)
    parts = [res.results[c]["y"] for c in range(8)]
    out = np.stack([parts[2 * b] + parts[2 * b + 1] for b in range(B)])
    return out.astype(np.float32), res


def kernel(x, w_qkv, w_proj):
    out, _ = run(get_nc(), prep_inputs(x, w_qkv, w_proj))
    return out
